# revision 9
# baseline (speedup 1.0000x reference)
"""GCN autoencoder (2x GCN layer + inner-product decoder) on 8 TRN2 NeuronCores.

Problem (full shapes):
    x [8192, 512] f32, w1 [512, 256] f32, w2 [256, 16] f32,
    edge_weight [262144] f32, row/col [262144] i32
    h1  = relu(segment_sum((x @ w1)[col] * ew, row, 8192))     # [8192, 256]
    z   = segment_sum((h1 @ w2)[col] * ew, row, 8192)          # [8192, 16]
    adj = z @ z.T                                              # [8192, 8192]

Strategy (node / destination-row sharding, 1024 rows per core):
  The COO graph is converted on host into a dense adjacency matrix
  A[r, c] = sum of edge_weight over edges (row=r, col=c), stored bf16;
  both GCN aggregations become dense matmuls against the SAME row shard
  A^T[:, own_dest] which is streamed once into SBUF and kept resident.

  The hidden dim (256) is split in two halves so the s2 AllGather overlaps
  compute (relu is elementwise per hidden dim, so
  s2 = relu(agg_lo) @ w2[:128] + relu(agg_hi) @ w2[128:] exactly):
    pass A (DMA-paced): s1 = x @ w1 (redundant); agg_lo accumulates over
      all 64 source chunks; agg_hi interleaved for chunks 0..SPLIT-1
      using spare PE cycles.
    AG-a: AllGather of partial s2_lo (32 KiB) -- overlapped by pass B.
    pass B (pure PE): agg_hi for chunks SPLIT..63 from SBUF-resident data.
    AG-b: AllGather of partial s2_hi; s2 = s2_lo + s2_hi.
    P5  z_c^T = s2^T @ ATr  (SBUF-resident reuse, [16, 1024])
    AG-z AllGather z -> z^T full, packed [128, 1024] (core-major).
    P7  adj_c = z_c @ z^T (K=16 matmuls, partition-sliced rhs), written
        bf16; the host converts to fp32 (~0.2% RMS, gate is 2e-2).
"""

import os
import sys

import numpy as np

if "/opt/trn_rl_repo" not in sys.path:
    sys.path.insert(0, "/opt/trn_rl_repo")

import ml_dtypes

import concourse.bass as bass
import concourse.mybir as mybir
import concourse.tile as tile
from concourse import bacc
from concourse.bass_utils import run_bass_kernel_spmd

N = 8192          # nodes
D_IN = 512        # input features
D_H = 256         # hidden features
D_Z = 16          # latent features
NCORES = 8
R = N // NCORES   # 1024 destination rows per core
P = 128
SPLIT = 32        # source chunks whose agg_hi matmuls run inside pass A

BF = mybir.dt.bfloat16
F32 = mybir.dt.float32

# stash for test harness introspection (exec_time_ns etc.)
LAST_RESULTS = None
_NC_CACHE = None


def _build_kernel(phases=7):
    nc = bacc.Bacc("TRN2", target_bir_lowering=False, debug=False,
                   num_devices=NCORES)

    xT = nc.dram_tensor("xT", [D_IN, N], BF, kind="ExternalInput").ap()
    w1 = nc.dram_tensor("w1", [D_IN, D_H], BF, kind="ExternalInput").ap()
    w2 = nc.dram_tensor("w2", [D_H, D_Z], BF, kind="ExternalInput").ap()
    # A^T row-shard (sources x own-destinations), partition-major:
    # ATr[p, k, r] = A^T[k*128 + p, core*R + r]
    ATr = nc.dram_tensor("ATr", [P, N // P, R], BF, kind="ExternalInput").ap()
    adjb = nc.dram_tensor("adjb", [R, N], BF, kind="ExternalOutput").ap()

    with tile.TileContext(nc) as tc:
        _body(tc, xT, w1, w2, ATr, adjb, phases)
    nc.compile()
    return nc


def _body(tc, xT, w1, w2, ATr, adjb, phases=7):
    nc = tc.nc
    KX = D_IN // P          # 4 k-chunks over input features
    KCH = N // P            # 64 source-node chunks
    DH_CH = D_H // P        # 2 chunks over hidden features
    RB = R // P             # 8 own row blocks

    xT_v = xT.rearrange("(k p) n -> p k n", p=P)                  # [128, 4, 8192]
    w1_v = w1.rearrange("(k p) n -> p k n", p=P)                  # [128, 4, 256]
    w2_v = w2.rearrange("(k p) n -> p k n", p=P)                  # [128, 2, 16]

    with (
        tc.tile_pool(name="const", bufs=1) as const,
        tc.tile_pool(name="persist", bufs=1) as persist,
        tc.tile_pool(name="xstream", bufs=2) as xstream,
        tc.tile_pool(name="s2rot", bufs=2) as s2rot,
        tc.tile_pool(name="outbuf", bufs=3) as outbuf,
        tc.tile_pool(name="psum_rot", bufs=2, space="PSUM") as psum_rot,
        tc.tile_pool(name="psum_acc", bufs=1, space="PSUM") as psum_acc,
        tc.tile_pool(name="dram", bufs=1, space="DRAM") as dram,
    ):
        # ---- constants ----
        w1s = const.tile([P, KX, D_H], BF)
        nc.sync.dma_start(w1s[:], w1_v[:])
        w2s = const.tile([P, DH_CH, D_Z], BF)
        nc.sync.dma_start(w2s[:], w2_v[:])

        # ---- persistent tiles ----
        atr_sb = persist.tile([P, KCH, R], BF)           # A^T shard, 128 KiB/part
        s1_sb = persist.tile([P, KCH, D_H], BF)          # x@w1      [8192, 256]
        h1T = persist.tile([P, DH_CH, R], BF)            # h1_c^T    [256, 1024]
        s2f_a = persist.tile([P, KCH, D_Z], BF)          # s2_lo full [8192, 16]
        s2f_b = persist.tile([P, KCH, D_Z], BF)          # s2_hi full [8192, 16]
        zT_c = persist.tile([D_Z, R], BF)                # z_c^T     [16, 1024]
        zT_sb = persist.tile([D_Z, NCORES, R], BF)       # z^T full  [16, 8192]

        # ========== pass A: s1 = x@w1 ; agg_lo (+ agg_hi for m < SPLIT) ===
        ph = [[psum_acc.tile([P, 512], F32, name=f"ph_{dh}_{nn}",
                             tag=f"ph_{dh}_{nn}")
               for nn in range(2)] for dh in range(2)]
        groups = [(0, 1), (1, 3)] + [(m, 4) for m in range(4, KCH, 4)]
        for (m0, gw) in groups:
            xts = xstream.tile([P, KX, 4 * P], BF, tag="xts")
            nc.sync.dma_start(xts[:, :, :gw * P],
                              xT_v[:, :, m0 * P:(m0 + gw) * P])
            nc.sync.dma_start(atr_sb[:, m0:m0 + gw, :],
                              ATr[:, m0:m0 + gw, :])
            for ml in range(gw):
                m = m0 + ml
                # P1: s1 chunk m = x_m @ w1  -> [128 nodes, 256]
                s1p = psum_rot.tile([P, D_H], F32, tag="psrot")
                for k in range(KX):
                    nc.tensor.matmul(
                        s1p[:], lhsT=xts[:, k, ml * P:(ml + 1) * P],
                        rhs=w1s[:, k], start=(k == 0), stop=(k == KX - 1))
                nc.vector.tensor_copy(s1_sb[:, m], s1p[:])
                # agg: accumulate h1_c^T over source chunk m
                dhs = (0, 1) if m < SPLIT else (0,)
                for dh in dhs:
                    for nn in range(2):
                        nc.tensor.matmul(
                            ph[dh][nn][:],
                            lhsT=s1_sb[:, m, dh * P:(dh + 1) * P],
                            rhs=atr_sb[:, m, nn * 512:(nn + 1) * 512],
                            start=(m == 0), stop=(m == KCH - 1))
        # relu of the low half; partial s2_lo; AllGather it
        for nn in range(2):
            nc.vector.tensor_scalar_max(
                h1T[:, 0, nn * 512:(nn + 1) * 512], ph[0][nn][:], 0.0)
        if phases < 3:
            return

        ag_a_in = dram.tile([R, D_Z], BF)
        ag_a_out = dram.tile([NCORES, R, D_Z], BF, addr_space="Shared")
        s2o_a = s2rot.tile([P, RB, D_Z], BF, tag="s2o")
        for ml in range(RB):
            s2p = psum_rot.tile([P, D_Z], F32, tag="psrot")
            nc.tensor.matmul(s2p[:], lhsT=h1T[:, 0, ml * P:(ml + 1) * P],
                             rhs=w2s[:, 0], start=True, stop=True)
            nc.vector.tensor_copy(s2o_a[:, ml], s2p[:])
        nc.sync.dma_start(ag_a_in[:].rearrange("(ml p) j -> p ml j", p=P),
                          s2o_a[:])
        nc.gpsimd.collective_compute(
            "AllGather", mybir.AluOpType.bypass,
            replica_groups=[list(range(NCORES))],
            ins=[ag_a_in[:].opt()], outs=[ag_a_out[:].opt()])
        nc.sync.dma_start(
            s2f_a[:], ag_a_out[:].rearrange("c (kk p) j -> p (c kk) j", p=P))

        # ========== pass B: agg_hi for chunks SPLIT..63 (pure PE) =========
        for m in range(SPLIT, KCH):
            for nn in range(2):
                nc.tensor.matmul(
                    ph[1][nn][:],
                    lhsT=s1_sb[:, m, P:2 * P],
                    rhs=atr_sb[:, m, nn * 512:(nn + 1) * 512],
                    start=False, stop=(m == KCH - 1))
        for nn in range(2):
            nc.vector.tensor_scalar_max(
                h1T[:, 1, nn * 512:(nn + 1) * 512], ph[1][nn][:], 0.0)

        ag_b_in = dram.tile([R, D_Z], BF)
        ag_b_out = dram.tile([NCORES, R, D_Z], BF, addr_space="Shared")
        s2o_b = s2rot.tile([P, RB, D_Z], BF, tag="s2o")
        for ml in range(RB):
            s2p = psum_rot.tile([P, D_Z], F32, tag="psrot")
            nc.tensor.matmul(s2p[:], lhsT=h1T[:, 1, ml * P:(ml + 1) * P],
                             rhs=w2s[:, 1], start=True, stop=True)
            nc.vector.tensor_copy(s2o_b[:, ml], s2p[:])
        nc.sync.dma_start(ag_b_in[:].rearrange("(ml p) j -> p ml j", p=P),
                          s2o_b[:])
        nc.gpsimd.collective_compute(
            "AllGather", mybir.AluOpType.bypass,
            replica_groups=[list(range(NCORES))],
            ins=[ag_b_in[:].opt()], outs=[ag_b_out[:].opt()])
        nc.sync.dma_start(
            s2f_b[:], ag_b_out[:].rearrange("c (kk p) j -> p (c kk) j", p=P))
        # s2 = s2_lo + s2_hi
        nc.vector.tensor_add(s2f_a[:], s2f_a[:], s2f_b[:])

        if phases < 5:
            return
        # ========== Phase 5: z_c^T = s2^T @ ATr (SBUF-resident reuse) =====
        pz = [psum_acc.tile([D_Z, 512], F32, name=f"pz_{nn}",
                            tag=f"ph_0_{nn}") for nn in range(2)]
        for k in range(KCH):
            for nn in range(2):
                nc.tensor.matmul(
                    pz[nn][:], lhsT=s2f_a[:, k],
                    rhs=atr_sb[:, k, nn * 512:(nn + 1) * 512],
                    start=(k == 0), stop=(k == KCH - 1))
        for nn in range(2):
            nc.vector.tensor_copy(zT_c[:, nn * 512:(nn + 1) * 512], pz[nn][:])

        if phases < 6:
            return
        # ========== AG-z: AllGather z -> z^T full, packed [c*16+i, r] =====
        ag_z_in = dram.tile([D_Z, R], BF)
        ag_z_out = dram.tile([NCORES, D_Z, R], BF, addr_space="Shared")
        nc.sync.dma_start(ag_z_in[:], zT_c[:])
        nc.gpsimd.collective_compute(
            "AllGather", mybir.AluOpType.bypass,
            replica_groups=[list(range(NCORES))],
            ins=[ag_z_in[:].opt()], outs=[ag_z_out[:].opt()])
        nc.sync.dma_start(zT_sb[:],
                          ag_z_out[:].rearrange("c i r -> i c r"))

        if phases < 7:
            return
        # ========== Phase 7: adj_c = z_c @ z^T (bf16 out) =================
        zT_flat = zT_sb[:].rearrange("i c r -> i (c r)")
        OWID = 1024  # output DMA chunk width (0.25 MiB per transfer)
        ptags = ["ph_0_0", "ph_0_1", "ph_1_0", "ph_1_1", "po_a", "po_b"]
        for mb in range(RB):
            for og in range(N // OWID):
                rowbuf = outbuf.tile([P, OWID], BF, tag="rowbuf")
                for ol in range(OWID // 512):
                    nb = og * (OWID // 512) + ol
                    po = psum_acc.tile(
                        [P, 512], F32, name=f"po_{mb}_{nb}",
                        tag=ptags[(mb * 16 + nb) % 6])
                    nc.tensor.matmul(
                        po[:], lhsT=zT_c[:, mb * P:(mb + 1) * P],
                        rhs=zT_flat[:, nb * 512:(nb + 1) * 512],
                        start=True, stop=True)
                    # split PSUM drains across DVE/ACT
                    dst = rowbuf[:, ol * 512:(ol + 1) * 512]
                    if ol % 2 == 0:
                        nc.vector.tensor_copy(dst, po[:])
                    else:
                        nc.scalar.copy(dst, po[:])
                nc.sync.dma_start(
                    adjb[mb * P:(mb + 1) * P, og * OWID:(og + 1) * OWID],
                    rowbuf[:])


def _get_nc():
    global _NC_CACHE
    phases = int(os.environ.get("BASS_KERNEL_PHASES", "7"))
    if _NC_CACHE is None or _NC_CACHE[0] != phases:
        _NC_CACHE = (phases, _build_kernel(phases))
    return _NC_CACHE[1]


def kernel(x, w1, w2, edge_weight, row, col):
    global LAST_RESULTS
    x = np.asarray(x, dtype=np.float32)
    w1 = np.asarray(w1, dtype=np.float32)
    w2 = np.asarray(w2, dtype=np.float32)
    edge_weight = np.asarray(edge_weight, dtype=np.float32)
    row = np.asarray(row, dtype=np.int64)
    col = np.asarray(col, dtype=np.int64)

    bf16 = ml_dtypes.bfloat16

    # Dense A^T: AT[c, r] = sum of edge_weight over edges with (row=r, col=c)
    # i.e. AT[source, dest]
    AT_dense = np.zeros((N, N), dtype=np.float32)
    np.add.at(AT_dense, (col, row), edge_weight)
    AT_bf = AT_dense.astype(bf16)

    xT_bf = np.ascontiguousarray(x.T).astype(bf16)
    w1_bf = w1.astype(bf16)
    w2_bf = w2.astype(bf16)

    in_maps = []
    for c in range(NCORES):
        # row shard: [src, own-dest] -> partition-major [128, 64, R]
        atr = AT_bf[:, c * R:(c + 1) * R]                 # [8192, 1024]
        atr = np.ascontiguousarray(
            atr.reshape(N // P, P, R).transpose(1, 0, 2))  # [128, 64, 1024]
        in_maps.append({
            "xT": xT_bf,
            "w1": w1_bf,
            "w2": w2_bf,
            "ATr": atr,
        })

    nc = _get_nc()
    print("kernel: launching on 8 cores", flush=True)
    res = run_bass_kernel_spmd(nc, in_maps, core_ids=list(range(NCORES)))
    print("kernel: run complete", flush=True)
    LAST_RESULTS = res
    adj = np.concatenate([res.results[c]["adjb"] for c in range(NCORES)],
                         axis=0)
    return np.ascontiguousarray(adj.astype(np.float32))


# revision 15
# speedup vs baseline: 1.1309x; 1.1309x over previous
"""GCN autoencoder (2x GCN layer + inner-product decoder) on 8 TRN2 NeuronCores.

Problem (full shapes):
    x [8192, 512] f32, w1 [512, 256] f32, w2 [256, 16] f32,
    edge_weight [262144] f32, row/col [262144] i32
    h1  = relu(segment_sum((x @ w1)[col] * ew, row, 8192))     # [8192, 256]
    z   = segment_sum((h1 @ w2)[col] * ew, row, 8192)          # [8192, 16]
    adj = z @ z.T                                              # [8192, 8192]

Strategy (node / destination-row sharding, 1024 rows per core):
  The COO graph is converted on host into a dense adjacency matrix
  A[r, c] = sum of edge_weight over edges (row=r, col=c), stored bf16;
  both GCN aggregations become dense matmuls against the SAME row shard
  A^T[:, own_dest] which is streamed once into SBUF and kept resident.
    P1+P2 pipelined per source chunk (keeps the PE p-state ramped):
        s1_m = x_m @ w1 ; h1_c^T += s1_m^T @ ATr_m
    P3  s2_c = relu(h1_c) @ w2              (local, [1024, 16])
    AG1 AllGather s2 (32 KiB payload); gather-in split over 4 DMA queues
        (the transposing descriptor runs ~25 GB/s, so parallelize it).
    P5  z_c^T = s2^T @ ATr  (SBUF-resident reuse, [16, 1024])
    AGz AllGather z -> z^T full [16, 8192].
    P7  adj_c = z_c @ z^T (K=16 matmuls) written as bf16 (the host converts
        to fp32; adds ~0.2% RMS, gate is 2e-2), 6-deep PSUM rotation,
        drains split DVE/ACT.
"""

import os
import sys

import numpy as np

if "/opt/trn_rl_repo" not in sys.path:
    sys.path.insert(0, "/opt/trn_rl_repo")

import ml_dtypes

import concourse.bass as bass
import concourse.mybir as mybir
import concourse.tile as tile
from concourse import bacc
from concourse.bass_utils import run_bass_kernel_spmd

N = 8192          # nodes
D_IN = 512        # input features
D_H = 256         # hidden features
D_Z = 16          # latent features
NCORES = 8
R = N // NCORES   # 1024 destination rows per core
P = 128

BF = mybir.dt.bfloat16
F32 = mybir.dt.float32

# stash for test harness introspection (exec_time_ns etc.)
LAST_RESULTS = None
_NC_CACHE = None


def _build_kernel(phases=7):
    nc = bacc.Bacc("TRN2", target_bir_lowering=False, debug=False,
                   num_devices=NCORES)

    xT = nc.dram_tensor("xT", [D_IN, N], BF, kind="ExternalInput").ap()
    w1 = nc.dram_tensor("w1", [D_IN, D_H], BF, kind="ExternalInput").ap()
    w2 = nc.dram_tensor("w2", [D_H, D_Z], BF, kind="ExternalInput").ap()
    # A^T row-shard (sources x own-destinations), partition-major:
    # ATr[p, k, r] = A^T[k*128 + p, core*R + r]
    ATr = nc.dram_tensor("ATr", [P, N // P, R], BF, kind="ExternalInput").ap()
    adjb = nc.dram_tensor("adjb", [R, N], BF, kind="ExternalOutput").ap()

    with tile.TileContext(nc) as tc:
        _body(tc, xT, w1, w2, ATr, adjb, phases)
    nc.compile()
    return nc


def _body(tc, xT, w1, w2, ATr, adjb, phases=7):
    nc = tc.nc
    KX = D_IN // P          # 4 k-chunks over input features
    KCH = N // P            # 64 source-node chunks
    DH_CH = D_H // P        # 2 chunks over hidden features
    RB = R // P             # 8 own row blocks

    xT_v = xT.rearrange("(k p) n -> p k n", p=P)                  # [128, 4, 8192]
    w1_v = w1.rearrange("(k p) n -> p k n", p=P)                  # [128, 4, 256]
    w2_v = w2.rearrange("(k p) n -> p k n", p=P)                  # [128, 2, 16]

    with (
        tc.tile_pool(name="const", bufs=1) as const,
        tc.tile_pool(name="persist", bufs=1) as persist,
        tc.tile_pool(name="xstream", bufs=2) as xstream,
        tc.tile_pool(name="s1rot", bufs=3) as s1rot,
        tc.tile_pool(name="outbuf", bufs=4) as outbuf,
        tc.tile_pool(name="psum_rot", bufs=2, space="PSUM") as psum_rot,
        tc.tile_pool(name="psum_acc", bufs=1, space="PSUM") as psum_acc,
        tc.tile_pool(name="dram", bufs=1, space="DRAM") as dram,
    ):
        # ---- constants ----
        w1s = const.tile([P, KX, D_H], BF)
        nc.sync.dma_start(w1s[:], w1_v[:])
        w2s = const.tile([P, DH_CH, D_Z], BF)
        nc.sync.dma_start(w2s[:], w2_v[:])

        # ---- persistent tiles ----
        atr_sb = persist.tile([P, KCH, R], BF)           # A^T shard, 128 KiB/part
        h1T = persist.tile([P, DH_CH, R], BF)            # h1_c^T    [256, 1024]
        s2o = persist.tile([P, RB, D_Z], BF)             # s2_c      [1024, 16]
        s2f = persist.tile([P, NCORES, RB, D_Z], BF)     # s2 full   [8192, 16]
        zT_c = persist.tile([D_Z, R], BF)                # z_c^T     [16, 1024]
        zT_sb = persist.tile([D_Z, NCORES, R], BF)       # z^T full  [16, 8192]

        # ========== P1+P2 pipelined: s1 = x@w1 ; h1_c^T += s1^T @ ATr =====
        ph = [[psum_acc.tile([P, 512], F32, name=f"ph_{dh}_{nn}",
                             tag=f"ph_{dh}_{nn}")
               for nn in range(2)] for dh in range(2)]
        groups = [(0, 1), (1, 3)] + [(m, 4) for m in range(4, KCH, 4)]
        for (m0, gw) in groups:
            xts = xstream.tile([P, KX, 4 * P], BF, tag="xts")
            nc.sync.dma_start(xts[:, :, :gw * P],
                              xT_v[:, :, m0 * P:(m0 + gw) * P])
            nc.sync.dma_start(atr_sb[:, m0:m0 + gw, :],
                              ATr[:, m0:m0 + gw, :])
            for ml in range(gw):
                m = m0 + ml
                # P1: s1 chunk m = x_m @ w1  -> [128 nodes, 256]
                s1p = psum_rot.tile([P, D_H], F32, tag="psrot")
                for k in range(KX):
                    nc.tensor.matmul(
                        s1p[:], lhsT=xts[:, k, ml * P:(ml + 1) * P],
                        rhs=w1s[:, k], start=(k == 0), stop=(k == KX - 1))
                s1c = s1rot.tile([P, D_H], BF, tag="s1c")
                nc.vector.tensor_copy(s1c[:], s1p[:])
                # P2: accumulate h1_c^T over source chunk m
                for dh in range(DH_CH):
                    for nn in range(2):
                        nc.tensor.matmul(
                            ph[dh][nn][:],
                            lhsT=s1c[:, dh * P:(dh + 1) * P],
                            rhs=atr_sb[:, m, nn * 512:(nn + 1) * 512],
                            start=(m == 0), stop=(m == KCH - 1))
        for dh in range(DH_CH):
            for nn in range(2):
                nc.vector.tensor_scalar_max(
                    h1T[:, dh, nn * 512:(nn + 1) * 512], ph[dh][nn][:], 0.0)

        if phases < 3:
            return
        # ========== Phase 3: s2_c = h1_c @ w2 (local) =====================
        for ml in range(RB):
            s2p = psum_rot.tile([P, D_Z], F32, tag="psrot")
            for dh in range(DH_CH):
                nc.tensor.matmul(
                    s2p[:], lhsT=h1T[:, dh, ml * P:(ml + 1) * P],
                    rhs=w2s[:, dh], start=(dh == 0), stop=(dh == DH_CH - 1))
            nc.vector.tensor_copy(s2o[:, ml], s2p[:])

        if phases < 4:
            return
        # ========== AG1: AllGather s2 -> s2 full ==========================
        # partition-major payload [p, kk, j] so the gather-in descriptor has
        # 256B runs; split over the two HWDGE queues (SP + ACT)
        ag1_in = dram.tile([P, RB, D_Z], BF)
        ag1_out = dram.tile([NCORES, P, RB, D_Z], BF, addr_space="Shared")
        nc.sync.dma_start(ag1_in[:], s2o[:])
        nc.gpsimd.collective_compute(
            "AllGather", mybir.AluOpType.bypass,
            replica_groups=[list(range(NCORES))],
            ins=[ag1_in[:].opt()], outs=[ag1_out[:].opt()])
        qs = [nc.sync, nc.scalar]
        for q in range(2):
            qs[q].dma_start(
                s2f[:, q * 4:(q + 1) * 4],
                ag1_out[:].rearrange("c p kk j -> p c kk j")
                [:, q * 4:(q + 1) * 4])

        if phases < 5:
            return
        # ========== Phase 5: z_c^T = s2^T @ ATr (SBUF-resident reuse) =====
        pz = [psum_acc.tile([D_Z, 512], F32, name=f"pz_{nn}",
                            tag=f"ph_0_{nn}") for nn in range(2)]
        for k in range(KCH):
            for nn in range(2):
                nc.tensor.matmul(
                    pz[nn][:], lhsT=s2f[:, k // RB, k % RB],
                    rhs=atr_sb[:, k, nn * 512:(nn + 1) * 512],
                    start=(k == 0), stop=(k == KCH - 1))
        for nn in range(2):
            nc.vector.tensor_copy(zT_c[:, nn * 512:(nn + 1) * 512], pz[nn][:])

        if phases < 6:
            return
        # ========== AGz: AllGather z -> z^T full ==========================
        ag_z_in = dram.tile([D_Z, R], BF)
        ag_z_out = dram.tile([NCORES, D_Z, R], BF, addr_space="Shared")
        nc.sync.dma_start(ag_z_in[:], zT_c[:])
        nc.gpsimd.collective_compute(
            "AllGather", mybir.AluOpType.bypass,
            replica_groups=[list(range(NCORES))],
            ins=[ag_z_in[:].opt()], outs=[ag_z_out[:].opt()])
        for q in range(2):
            qs[q].dma_start(
                zT_sb[:, q * 4:(q + 1) * 4],
                ag_z_out[:].rearrange("c i r -> i c r")[:, q * 4:(q + 1) * 4])

        if phases < 7:
            return
        # ========== Phase 7: adj_c = z_c @ z^T (bf16 out) =================
        zT_flat = zT_sb[:].rearrange("i c r -> i (c r)")
        OWID = 2048  # output DMA chunk width (0.5 MiB per transfer)
        ptags = ["ph_0_0", "ph_0_1", "ph_1_0", "ph_1_1", "po_a", "po_b"]
        for mb in range(RB):
            for og in range(N // OWID):
                rowbuf = outbuf.tile([P, OWID], BF, tag="rowbuf")
                for ol in range(OWID // 512):
                    nb = og * (OWID // 512) + ol
                    po = psum_acc.tile(
                        [P, 512], F32, name=f"po_{mb}_{nb}",
                        tag=ptags[(mb * 16 + nb) % 6])
                    nc.tensor.matmul(
                        po[:], lhsT=zT_c[:, mb * P:(mb + 1) * P],
                        rhs=zT_flat[:, nb * 512:(nb + 1) * 512],
                        start=True, stop=True)
                    # split PSUM drains across DVE/ACT
                    dst = rowbuf[:, ol * 512:(ol + 1) * 512]
                    if ol % 2 == 0:
                        nc.vector.tensor_copy(dst, po[:])
                    else:
                        nc.scalar.copy(dst, po[:])
                nc.sync.dma_start(
                    adjb[mb * P:(mb + 1) * P, og * OWID:(og + 1) * OWID],
                    rowbuf[:])


def _get_nc():
    global _NC_CACHE
    phases = int(os.environ.get("BASS_KERNEL_PHASES", "7"))
    if _NC_CACHE is None or _NC_CACHE[0] != phases:
        _NC_CACHE = (phases, _build_kernel(phases))
    return _NC_CACHE[1]


def kernel(x, w1, w2, edge_weight, row, col):
    global LAST_RESULTS
    x = np.asarray(x, dtype=np.float32)
    w1 = np.asarray(w1, dtype=np.float32)
    w2 = np.asarray(w2, dtype=np.float32)
    edge_weight = np.asarray(edge_weight, dtype=np.float32)
    row = np.asarray(row, dtype=np.int64)
    col = np.asarray(col, dtype=np.int64)

    bf16 = ml_dtypes.bfloat16

    # Dense A^T: AT[c, r] = sum of edge_weight over edges with (row=r, col=c)
    # i.e. AT[source, dest]
    AT_dense = np.zeros((N, N), dtype=np.float32)
    np.add.at(AT_dense, (col, row), edge_weight)
    AT_bf = AT_dense.astype(bf16)

    xT_bf = np.ascontiguousarray(x.T).astype(bf16)
    w1_bf = w1.astype(bf16)
    w2_bf = w2.astype(bf16)

    in_maps = []
    for c in range(NCORES):
        # row shard: [src, own-dest] -> partition-major [128, 64, R]
        atr = AT_bf[:, c * R:(c + 1) * R]                 # [8192, 1024]
        atr = np.ascontiguousarray(
            atr.reshape(N // P, P, R).transpose(1, 0, 2))  # [128, 64, 1024]
        in_maps.append({
            "xT": xT_bf,
            "w1": w1_bf,
            "w2": w2_bf,
            "ATr": atr,
        })

    nc = _get_nc()
    print("kernel: launching on 8 cores", flush=True)
    res = run_bass_kernel_spmd(nc, in_maps, core_ids=list(range(NCORES)))
    print("kernel: run complete", flush=True)
    LAST_RESULTS = res
    adj = np.concatenate([res.results[c]["adjb"] for c in range(NCORES)],
                         axis=0)
    return np.ascontiguousarray(adj.astype(np.float32))
